# revision 16
# baseline (speedup 1.0000x reference)
"""Trainium2 8-core GCN kernel (2-layer GCNConv + linear head + softmax).

Strategy (node/row partitioning, dense normalized adjacency):
  - Host: build Ahat = D^-1/2 (A+I) D^-1/2 as a dense fp8-e4m3 matrix, padded
    from 10000 to 10240 nodes; core k owns node rows [k*1280, (k+1)*1280).
  - Device, per core k (all matmuls fp8-e4m3 DoubleRow, fp32 accumulate):
      t1     = x @ W1 for ALL nodes (replicated GEMM; cheaper than the
               all-gather + reload stall it replaces)
      h1T_k  = relu(t1^T Ahat^T[:,k] + b1)    (transposed SpMM -> [512,1280])
      t2_k   = (h1T_k)^T @ W2                 (h1T is directly the lhsT)
      t2     = AllGather(t2_k) in 3 chunks
      h2T_k  = relu(t2^T Ahat^T[:,k] + b2)
      out_k  = softmax(h2T_k^T @ Wout + bout) ([1280, 16] f32)
  - Host: concatenate core outputs, trim padding to [10000, 16].

v2 schedule notes (vs the first working version):
  - SpMM1 runs as THREE dst-chunk passes (c1=cols 512:1024, c2=1024:1280,
    c0=0:512), each immediately followed by its GEMM2 m-tiles + AllGather
    launch, so the collective pipeline starts right after the first pass
    instead of after the whole layer.  SpMM2 runs as TWO passes (c1+c2,
    then c0) consuming j-pairs in AG-arrival order [c1, c2, c0].
  - A's c1+c2 column chunks (7.9 MB/core) stay resident in SBUF across
    both layers; only the c0 chunk is re-streamed in layer 2.  Layer-2's
    final pass reads A entirely from SBUF.
  - GEMM1's PSUM->SBUF casts alternate between DVE and Pool engines (the
    casts, not the matmuls, gated GEMM1 at ~690ns/tile).  SpMM evacs are
    likewise split, except where a pending collective could block the
    Pool queue.
  - x is loaded in 4-j-tile batches; t2 chunks load with one DMA per
    (chunk, peer core) via a rearranged AP; DMA queues are kept separate
    (sync: x/t2/ag_in/out, scalar: A stream, gpsimd: collectives only).
  - A tiny warmup AllGather runs during GEMM1 to absorb the first-op
    collective overhead before AG1 hits the critical path.

All matmuls use perf_mode=DoubleRow (256 contraction rows per matmul):
lhsT/rhs are [128, 2, free] pair tiles, element [p, q] = contraction row
q*128+p.  The transposed SpMM (z^T = t^T A^T) makes each layer's
activation land in [feature, node] layout, exactly the lhsT the next
GEMM needs -- no on-device transposes anywhere.
"""

import contextlib
import ctypes
import sys
import types

import ml_dtypes
import numpy as np

import concourse.bass as bass
import concourse.mybir as mybir
import concourse.tile as tile
from concourse.bass_utils import run_bass_kernel_spmd

BF16 = ml_dtypes.bfloat16
FP8 = ml_dtypes.float8_e4m3

N_CORES = 8
N_NODES = 10000
F_IN = 512
F_HID = 512
N_CLASSES = 16
NP = 10240            # padded node count (80 * 128)
R = NP // N_CORES     # 1280 rows per core
P = 128
NJ = NP // P          # 80 contraction chunks
NJ4 = NJ // 4         # 20 four-tile x groups
NJP = NJ // 2         # 40 DoubleRow contraction pairs
NM = R // P           # 10 row tiles per core
NF = F_HID // P       # 4 feature tiles
NFP = NF // 2         # 2 feature pairs

# dst-column chunks of this core's 1280 columns; AG launch order c1, c2, c0
CH_OFF = {"c0": 0, "c1": 512, "c2": 1024}
CH_W = {"c0": 512, "c1": 512, "c2": 256}
CH_M = {"c0": range(0, 4), "c1": range(4, 8), "c2": range(8, 10)}
CH_NT = {"c0": 4, "c1": 4, "c2": 2}     # j-tiles per core per chunk

_NTFF_HOOK_INSTALLED = False


def install_ntff_hook():
    """bass_utils' trace=True path wants antenv.axon_hooks; this container
    doesn't ship it, so provide the same ctypes hook trn_boot would."""
    global _NTFF_HOOK_INSTALLED
    if _NTFF_HOOK_INSTALLED:
        return
    _NTFF_HOOK_INSTALLED = True
    try:
        lib = ctypes.CDLL("/opt/axon/libaxon_pjrt.so")
        if not hasattr(lib, "axon_start_nrt_profile"):
            return
    except OSError:
        return
    lib.axon_start_nrt_profile.argtypes = [
        ctypes.POINTER(ctypes.c_int64),
        ctypes.c_size_t,
    ]
    lib.axon_start_nrt_profile.restype = ctypes.c_int64
    lib.axon_stop_nrt_profile.argtypes = [ctypes.c_char_p]
    lib.axon_stop_nrt_profile.restype = ctypes.c_int64

    @contextlib.contextmanager
    def _hook(output_dir, device_ids):
        import jax

        jax.devices()
        if device_ids:
            ids = (ctypes.c_int64 * len(device_ids))(*device_ids)
            rc = lib.axon_start_nrt_profile(ids, len(device_ids))
        else:
            rc = lib.axon_start_nrt_profile(None, 0)
        if rc != 0:
            raise RuntimeError(f"axon_start_nrt_profile rc={rc}")
        try:
            yield
        finally:
            n = lib.axon_stop_nrt_profile(str(output_dir).encode())
            print(f"ntff profile: {n} file(s) -> {output_dir}", file=sys.stderr)

    import antenv

    mod = types.ModuleType("antenv.axon_hooks")
    mod.get_axon_ntff_profile_hook = lambda: _hook
    mod.set_axon_ntff_profile_hook = lambda h: None
    sys.modules["antenv.axon_hooks"] = mod
    antenv.axon_hooks = mod


def split_drain_waits(nc):
    """This walrus build allows only ONE sync-wait per lowered instruction
    (CTRL and pseudo-DMA structs assert on more).  Tile's wait-assignment can
    attach several; keep the last wait on the instruction and move the rest
    onto preceding single-wait NoOps on the same engine stream (waits are
    monotonic >= conditions, so enforcing them earlier in program order on
    the same engine is equivalent)."""
    for f in nc.m.functions:
        for bb in f.blocks:
            insts = bb.instructions
            i = 0
            while i < len(insts):
                inst = insts[i]
                si = getattr(inst, "sync_info", None)
                if si is not None and si.on_wait and len(si.on_wait) > 1:
                    waits = list(si.on_wait)
                    si.on_wait = [waits[-1]]
                    for j, w in enumerate(waits[:-1]):
                        pre = mybir.InstNoOp(
                            name=f"{inst.name}-presync-{j}",
                            engine=inst.engine,
                            ins=[],
                            outs=[],
                            sync_info=mybir.SyncInfo(on_wait=[w], on_update=[]),
                        )
                        insts.insert(i + j, pre)
                        nc.register_instruction(pre, overwrite=True)
                    i += len(waits) - 1
                i += 1


def build_gcn(nc):
    """Emit the SPMD GCN program (identical on every core; per-core data)."""
    f32 = mybir.dt.float32
    bf16 = mybir.dt.bfloat16
    fp8 = mybir.dt.float8e4
    DR = mybir.MatmulPerfMode.DoubleRow
    rg = [list(range(N_CORES))]

    # I/O (per-core shards; same names on every core)
    # xTt84[g, p, q, cq, m] = x[(4g+q)*128+m, cq*128+p]  (fp8)
    xTt84 = nc.declare_dram_parameter("xTt84", [NJ4, P, 4, NF, P], fp8, isOutput=False)
    # ATdr[jp, p, q, m] = AhatT[jp*256 + q*128 + p, k*R + m]  (fp8 pairs)
    ATdr = nc.declare_dram_parameter("ATdr", [NJP, P, 2, R], fp8, isOutput=False)
    # W pair layouts: W*p8[t, p, q, n] = W[(2t+q)*128 + p, n]
    W1p = nc.declare_dram_parameter("W1p", [NFP, P, 2, F_HID], fp8, isOutput=False)
    W2p = nc.declare_dram_parameter("W2p", [NFP, P, 2, F_HID], fp8, isOutput=False)
    Woutp = nc.declare_dram_parameter("Woutp", [NFP, P, 2, N_CLASSES], fp8, isOutput=False)
    bcols = nc.declare_dram_parameter("bcols", [P, 2 * NF], f32, isOutput=False)
    bout = nc.declare_dram_parameter("bout", [1, N_CLASSES], bf16, isOutput=False)
    out = nc.declare_dram_parameter("out", [R, N_CLASSES], f32, isOutput=True)

    # layer-2 collective bounce buffers (internal DRAM), 3 chunks, fp8
    ag_in = nc.dram_tensor("ag_in", [R, F_HID], fp8)
    ag_out = {
        c: nc.dram_tensor(
            f"ag_out_{c}", [N_CORES * CH_NT[c], P, F_HID], fp8, addr_space="Shared"
        )
        for c in ("c0", "c1", "c2")
    }
    ag_wu_in = nc.dram_tensor("ag_wu_in", [P, 64], fp8)
    ag_wu_out = nc.dram_tensor("ag_wu_out", [N_CORES * P, 64], fp8, addr_space="Shared")

    with tile.TileContext(nc) as tc:
        with (
            tc.tile_pool(name="const", bufs=1) as cpool,
            tc.tile_pool(name="ares", bufs=1) as apool,
            tc.tile_pool(name="tfull", bufs=1) as tpool,
            tc.tile_pool(name="hT", bufs=1) as hpool,
            tc.tile_pool(name="work", bufs=4) as wpool,
            tc.tile_pool(name="evac", bufs=4) as epool,
            tc.tile_pool(name="sm", bufs=4) as spool,
            tc.tile_pool(name="psum", bufs=1, space="PSUM") as ppool,
        ):
            # ---- GEMM1 constants (needed immediately) ----
            W1_sb = [cpool.tile([P, 2, F_HID], fp8, tag=f"W1{t}", name=f"W1{t}") for t in range(NFP)]
            for t in range(NFP):
                nc.sync.dma_start(out=W1_sb[t][:], in_=W1p[t, :, :, :])

            # warmup AG feed (trigger is emitted after GEMM1's pool casts)
            wu_sb = cpool.tile([P, 64], fp8, tag="wu", name="wu")
            nc.vector.memset(wu_sb[:], 0.0)
            nc.sync.dma_start(out=ag_wu_in[:, :], in_=wu_sb[:])

            # persistent activation tiles
            # layer 1: j-PAIR tiles  t1p[jp][p, q, f] = t1[jp*256+q*128+p... ]
            t1p = [
                tpool.tile([P, 2, F_HID], fp8, tag=f"t1_{jp}", name=f"t1_{jp}")
                for jp in range(NJP)
            ]
            # layer 2: per (chunk, peer core) tiles holding that core's
            # chunk j-tiles: t2c[c][r][p, i, f] = t2[(r*NM + base + i)*128 + p, f]
            t2c = {
                c: [
                    tpool.tile([P, CH_NT[c], F_HID], fp8, tag=f"t2{c}_{r}", name=f"t2{c}_{r}")
                    for r in range(N_CORES)
                ]
                for c in ("c0", "c1", "c2")
            }
            # hT as fp8 feature-pair tiles: hp[layer][t][p, q, m], ft = 2t+q
            hp = [
                [hpool.tile([P, 2, R], fp8, tag=f"h{la}p{t}", name=f"h{la}p{t}") for t in range(NFP)]
                for la in range(2)
            ]
            # resident A chunks (loaded once up front, reused in both layers);
            # the loads have no deps, so the scalar DMA ring streams them all
            # during GEMM1, well ahead of the SpMM passes that consume them.
            atc1 = [
                apool.tile([P, 2, CH_W["c1"]], fp8, tag=f"ac1_{jp}", name=f"ac1_{jp}")
                for jp in range(NJP)
            ]
            atc2 = [
                apool.tile([P, 2, CH_W["c2"]], fp8, tag=f"ac2_{jp}", name=f"ac2_{jp}")
                for jp in range(NJP)
            ]
            def load_res(eng, tiles, chunk, jp):
                off, w = CH_OFF[chunk], CH_W[chunk]
                eng.dma_start(out=tiles[jp][:], in_=ATdr[jp, :, :, off:off + w])

            # c2 resident loads go on the otherwise-idle gpsimd ring (bulk
            # issue; its queue has nothing else until the warmup AG, so
            # descriptor-ring backpressure is harmless)
            for jp in range(NJP):
                load_res(nc.gpsimd, atc2, "c2", jp)

            # ---- layer 1: replicated GEMM1 (fp8 DoubleRow) ----
            for g in range(NJ4):
                xt4 = wpool.tile([P, 4, NF, P], fp8, tag="xt4", name="xt4")
                nc.sync.dma_start(out=xt4[:], in_=xTt84[g, :, :, :, :])
                for q in range(4):
                    j = 4 * g + q
                    # 8-bank rotation: with 4 banks the mm->cast->mm chain
                    # (two cross-engine semaphore hops per lap) paces GEMM1
                    ps = ppool.tile([P, F_HID], f32, tag=f"sp{j % 8}", name=f"g1ps{j % 8}")
                    for t in range(NFP):
                        nc.tensor.matmul(
                            out=ps[:],
                            lhsT=xt4[:, q, 2 * t:2 * t + 2, :],
                            rhs=W1_sb[t][:, :, :],
                            start=(t == 0),
                            stop=(t == NFP - 1),
                            perf_mode=DR,
                        )
                    # casts split across DVE/Act; one resident-A load per j,
                    # interleaved so neither DMA ring backs up its engine
                    # queue (a bulk 40-load block stalls the queue -- and the
                    # casts behind it -- on descriptor-ring space)
                    if j % 2 == 0:
                        nc.vector.tensor_copy(out=t1p[j // 2][:, j % 2, :], in_=ps[:])
                    else:
                        nc.scalar.activation(
                            out=t1p[j // 2][:, j % 2, :], in_=ps[:],
                            func=mybir.ActivationFunctionType.Copy,
                        )
                        load_res(nc.scalar, atc1, "c1", j // 2)

            # warmup collective: absorbs first-op overhead during GEMM1
            nc.gpsimd.collective_compute(
                "AllGather",
                mybir.AluOpType.bypass,
                replica_groups=rg,
                ins=[ag_wu_in[:, :].opt()],
                outs=[ag_wu_out[:, :].opt()],
            )

            # ---- deferred constants (needed after GEMM1 starts) ----
            W2_sb = [cpool.tile([P, 2, F_HID], fp8, tag=f"W2{t}", name=f"W2{t}") for t in range(NFP)]
            for t in range(NFP):
                nc.sync.dma_start(out=W2_sb[t][:], in_=W2p[t, :, :, :])
            Wout_sb = [cpool.tile([P, 2, N_CLASSES], fp8, tag=f"Wo{t}", name=f"Wo{t}") for t in range(NFP)]
            for t in range(NFP):
                nc.sync.dma_start(out=Wout_sb[t][:], in_=Woutp[t, :, :, :])
            bcols_sb = cpool.tile([P, 2 * NF], f32, tag="bcols", name="bcols")
            nc.sync.dma_start(out=bcols_sb[:], in_=bcols[:, :])
            bout_sb = cpool.tile([1, N_CLASSES], bf16, tag="bout", name="bout")
            nc.sync.dma_start(out=bout_sb[:], in_=bout[:, :])
            ones_sb = cpool.tile([1, P], bf16, tag="ones", name="ones")
            nc.vector.memset(ones_sb[:], 1.0)

            # ---- helpers ----
            def src1(jp):
                return t1p[jp]

            def src2(jp):
                r, k = jp // 5, jp % 5
                if k < 2:
                    return t2c["c0"][r][:, 2 * k:2 * k + 2, :]
                if k < 4:
                    return t2c["c1"][r][:, 2 * (k - 2):2 * (k - 2) + 2, :]
                return t2c["c2"][r][:, 0:2, :]

            def a_res(tiles):
                return lambda jp: tiles[jp]

            AT0_BUFS = 12
            at0_pending = {}

            def a_stream_load(jp):
                at = wpool.tile(
                    [P, 2, CH_W["c0"]], fp8, tag="at0", name="at0", bufs=AT0_BUFS
                )
                nc.scalar.dma_start(
                    out=at[:],
                    in_=ATdr[jp, :, :, CH_OFF["c0"]:CH_OFF["c0"] + CH_W["c0"]],
                )
                at0_pending[jp] = at

            def a_stream(jp_order):
                """Streamed c0 A tiles, prefetched AT0_BUFS deep.  The load
                for jp_order[i+BUFS] is emitted via post() AFTER jp_order[i]'s
                matmuls so the ring-buffer WAR (writer of slot i+BUFS after
                readers of slot i) follows program order."""
                for jp in jp_order[:AT0_BUFS]:
                    a_stream_load(jp)

                def get(jp):
                    return at0_pending.pop(jp)

                def post(idx):
                    k = idx + AT0_BUFS
                    if k < len(jp_order):
                        a_stream_load(jp_order[k])

                return get, post

            def spmm_pass(name, layer, src_fn, chunks, jp_order, post=None):
                """One fp8 DoubleRow accumulation sweep over the given dst
                chunks of hT[layer] = relu(t^T A^T + b).
                chunks: list of (chunk_key, bank_base, a_fn)."""
                pst = {}
                for (ck, bb, _a) in chunks:
                    for f in range(NF):
                        pst[(ck, f)] = ppool.tile(
                            [P, CH_W[ck]], f32, tag=f"sp{bb + f}", name=f"{name}_{bb + f}"
                        )
                last = len(jp_order) - 1
                for idx, jp in enumerate(jp_order):
                    ats = {ck: a_fn(jp) for (ck, bb, a_fn) in chunks}
                    src = src_fn(jp)
                    for f in range(NF):
                        for (ck, bb, _a) in chunks:
                            nc.tensor.matmul(
                                out=pst[(ck, f)][:],
                                lhsT=src[:, :, f * P:(f + 1) * P],
                                rhs=ats[ck][:, :, :],
                                start=(idx == 0),
                                stop=(idx == last),
                                perf_mode=DR,
                            )
                    if post is not None:
                        post(idx)
                # evacuate: relu(psum + b) -> fp8 pair tiles; f-tile ft=2t+q
                # (split across DVE and Activation so neither engine gates the
                # downstream GEMM2 + AllGather launch)
                for (ck, bb, _a) in chunks:
                    off, w = CH_OFF[ck], CH_W[ck]
                    for f in range(NF):
                        bc = bcols_sb[:, layer * NF + f:layer * NF + f + 1]
                        if f % 2 == 0:
                            nc.vector.tensor_scalar(
                                out=hp[layer][f // 2][:, f % 2, off:off + w],
                                in0=pst[(ck, f)][:],
                                scalar1=bc,
                                scalar2=0.0,
                                op0=mybir.AluOpType.add,
                                op1=mybir.AluOpType.max,
                            )
                        else:
                            nc.scalar.activation(
                                out=hp[layer][f // 2][:, f % 2, off:off + w],
                                in_=pst[(ck, f)][:],
                                func=mybir.ActivationFunctionType.Relu,
                                bias=bc,
                            )

            def gemm2_tiles(ms):
                """t2_k rows for m-tiles `ms` staged into ag_in (as fp8)."""
                for m in ms:
                    ps = ppool.tile([P, F_HID], f32, tag=f"sp{m % 4}", name=f"g2ps{m % 4}")
                    for t in range(NFP):
                        nc.tensor.matmul(
                            out=ps[:],
                            lhsT=hp[0][t][:, :, m * P:(m + 1) * P],
                            rhs=W2_sb[t][:, :, :],
                            start=(t == 0),
                            stop=(t == NFP - 1),
                            perf_mode=DR,
                        )
                    ev = epool.tile([P, F_HID], fp8, tag="g2ev", name="g2ev")
                    nc.vector.tensor_copy(out=ev[:], in_=ps[:])
                    nc.sync.dma_start(out=ag_in[m * P:(m + 1) * P, :], in_=ev[:])

            def ag_chunk(c):
                off = CH_OFF[c]
                rows = CH_W[c]
                nc.gpsimd.collective_compute(
                    "AllGather",
                    mybir.AluOpType.bypass,
                    replica_groups=rg,
                    ins=[ag_in[off:off + rows, :].opt()],
                    outs=[ag_out[c][:, :, :].opt()],
                )

            def load_t2_chunk(c):
                nt = CH_NT[c]
                for r in range(N_CORES):
                    nc.sync.dma_start(
                        out=t2c[c][r][:, :, :],
                        in_=ag_out[c][r * nt:(r + 1) * nt, :, :].rearrange(
                            "a b c -> b a c"
                        ),
                    )

            def head_tiles(ms):
                """logits + softmax for m-tiles `ms` of this core."""
                for m in ms:
                    ps = ppool.tile([P, N_CLASSES], f32, tag=f"sp{m % 4}", name=f"hps{m % 4}")
                    for t in range(NFP):
                        nc.tensor.matmul(
                            out=ps[:],
                            lhsT=hp[1][t][:, :, m * P:(m + 1) * P],
                            rhs=Wout_sb[t][:, :, :],
                            start=(t == 0),
                            stop=False,
                            perf_mode=DR,
                        )
                    nc.tensor.matmul(
                        out=ps[:],
                        lhsT=ones_sb[:, 0:P],
                        rhs=bout_sb[:],
                        start=False,
                        stop=True,
                    )
                    negmax = spool.tile([P, 1], f32, tag="negmax", name="negmax")
                    nc.vector.tensor_reduce(
                        out=negmax[:], in_=ps[:], axis=mybir.AxisListType.X,
                        op=mybir.AluOpType.max, negate=True,
                    )
                    ex = spool.tile([P, N_CLASSES], f32, tag="ex", name="ex")
                    nc.scalar.activation(
                        out=ex[:], in_=ps[:],
                        func=mybir.ActivationFunctionType.Exp,
                        bias=negmax[:, 0:1],
                    )
                    ssum = spool.tile([P, 1], f32, tag="ssum", name="ssum")
                    nc.vector.tensor_reduce(
                        out=ssum[:], in_=ex[:], axis=mybir.AxisListType.X,
                        op=mybir.AluOpType.add,
                    )
                    rinv = spool.tile([P, 1], f32, tag="rinv", name="rinv")
                    nc.vector.reciprocal(out=rinv[:], in_=ssum[:])
                    prob = spool.tile([P, N_CLASSES], f32, tag="prob", name="prob")
                    nc.vector.tensor_scalar_mul(prob[:], ex[:], rinv[:, 0:1])
                    nc.sync.dma_start(out=out[m * P:(m + 1) * P, :], in_=prob[:])

            natural = list(range(NJP))
            # SpMM2 consumes j-pairs in AG-arrival order: c1, c2, c0
            order2 = (
                [5 * r + k for r in range(N_CORES) for k in (2, 3)]
                + [5 * r + 4 for r in range(N_CORES)]
                + [5 * r + k for r in range(N_CORES) for k in (0, 1)]
            )
            assert sorted(order2) == natural

            # ---- layer 1 SpMM: 3 chunk passes, GEMM2 + AG after each ----
            spmm_pass("s1c1", 0, src1, [("c1", 4, a_res(atc1))], natural)
            gemm2_tiles(CH_M["c1"])
            ag_chunk("c1")
            g1, p1 = a_stream(natural)
            spmm_pass("s1c2", 0, src1, [("c2", 0, a_res(atc2))], natural)
            gemm2_tiles(CH_M["c2"])
            ag_chunk("c2")
            load_t2_chunk("c1")
            spmm_pass("s1c0", 0, src1, [("c0", 4, g1)], natural, post=p1)
            gemm2_tiles(CH_M["c0"])
            ag_chunk("c0")
            load_t2_chunk("c2")
            load_t2_chunk("c0")

            # ---- layer 2 SpMM: c1+c2 pass (resident A), then c0 pass ----
            g2, p2 = a_stream(order2)
            spmm_pass(
                "s2a", 1, src2,
                [("c1", 4, a_res(atc1)), ("c2", 0, a_res(atc2))],
                order2,
            )
            head_tiles(list(CH_M["c1"]) + list(CH_M["c2"]))
            spmm_pass("s2b", 1, src2, [("c0", 4, g2)], order2, post=p2)
            head_tiles(CH_M["c0"])

    return nc


def build_inputs(x, edge_index, W1, b1, W2, b2, Wout, bout):
    """Host-side graph preprocessing + per-core shard construction."""
    x = np.asarray(x)
    ei = np.asarray(edge_index)
    n = N_NODES
    src = np.concatenate([ei[0], np.arange(n, dtype=np.int64)])
    dst = np.concatenate([ei[1], np.arange(n, dtype=np.int64)])
    deg = np.bincount(dst, minlength=n).astype(np.float32)
    dinv = 1.0 / np.sqrt(deg)
    normv = (dinv[src] * dinv[dst]).astype(np.float32)

    # dense Ahat^T, padded:  AhatT[src, dst] = norm  (duplicate edges sum)
    AhatT = np.zeros((NP, NP), dtype=np.float32)
    np.add.at(AhatT, (src, dst), normv)
    # DoubleRow pair-interleave: ATdr[jp, p, q, :] = AhatT[jp*256+q*128+p, :]
    ATdr = np.ascontiguousarray(
        AhatT.reshape(NJP, 2, P, NP).transpose(0, 2, 1, 3)
    ).astype(FP8)

    xp = np.zeros((NP, F_IN), dtype=np.float32)
    xp[:n] = x
    # xTt84[g, p, q, cq, m] = x[(4g+q)*128+m, cq*128+p]
    xTt84 = np.ascontiguousarray(
        xp.reshape(NJ4, 4, P, NF, P).transpose(0, 4, 1, 3, 2)
    ).astype(FP8)

    def wpairs(W):
        W = np.asarray(W, np.float32)
        # [t, p, q, n] = W[(2t+q)*128+p, n]
        return np.ascontiguousarray(
            W.reshape(NFP, 2, P, W.shape[1]).transpose(0, 2, 1, 3)
        ).astype(FP8)

    W1b = wpairs(W1)
    W2b = wpairs(W2)
    Woutb = wpairs(Wout)
    boutb = np.asarray(bout).reshape(1, N_CLASSES).astype(BF16)
    # biases as per-partition columns: bcols[:, l*NF + f] = b_l[f*128:(f+1)*128]
    bcols = np.stack(
        [np.asarray(b1).reshape(NF, P), np.asarray(b2).reshape(NF, P)], 0
    ).reshape(2 * NF, P).T.astype(np.float32)
    bcols = np.ascontiguousarray(bcols)

    in_maps = []
    for k in range(N_CORES):
        sl = slice(k * R, (k + 1) * R)
        in_maps.append({
            "xTt84": xTt84,
            "ATdr": np.ascontiguousarray(ATdr[:, :, :, sl]),
            "W1p": W1b,
            "W2p": W2b,
            "Woutp": Woutb,
            "bcols": bcols,
            "bout": boutb,
        })
    return in_maps


_CACHED = {}


def _get_program():
    if "nc" not in _CACHED:
        nc = bass.Bass(num_devices=N_CORES)
        build_gcn(nc)
        split_drain_waits(nc)
        _CACHED["nc"] = nc
    return _CACHED["nc"]


def kernel(x, edge_index, W1, b1, W2, b2, Wout, bout, trace=False):
    install_ntff_hook()
    nc = _get_program()
    in_maps = build_inputs(x, edge_index, W1, b1, W2, b2, Wout, bout)
    res = run_bass_kernel_spmd(
        nc, in_maps, core_ids=list(range(N_CORES)), trace=trace
    )
    out = np.concatenate([res.results[k]["out"] for k in range(N_CORES)], 0)
    kernel.last_exec_time_ns = res.exec_time_ns
    kernel.last_results = res
    return out[:N_NODES].astype(np.float32)


kernel.last_exec_time_ns = None
kernel.last_results = None


# revision 23
# speedup vs baseline: 1.0241x; 1.0241x over previous
"""Trainium2 8-core GCN kernel (2-layer GCNConv + linear head + softmax).

Strategy (node/row partitioning, dense normalized adjacency):
  - Host: build Ahat = D^-1/2 (A+I) D^-1/2 as a dense fp8-e4m3 matrix, padded
    from 10000 to 10240 nodes; core k owns node rows [k*1280, (k+1)*1280).
  - Device, per core k (all matmuls fp8-e4m3 DoubleRow, fp32 accumulate):
      t1     = x @ W1 for ALL nodes (replicated GEMM; cheaper than the
               all-gather + reload stall it replaces)
      h1T_k  = relu(t1^T Ahat^T[:,k] + b1)    (transposed SpMM -> [512,1280])
      t2_k   = (h1T_k)^T @ W2                 (h1T is directly the lhsT)
      t2     = AllGather(t2_k) in 3 chunks
      h2T_k  = relu(t2^T Ahat^T[:,k] + b2)
      out_k  = softmax(h2T_k^T @ Wout + bout) ([1280, 16] f32)
  - Host: concatenate core outputs, trim padding to [10000, 16].

v2 schedule notes (vs the first working version):
  - SpMM1 runs as THREE dst-chunk passes (c1=cols 512:1024, c2=1024:1280,
    c0=0:512), each immediately followed by its GEMM2 m-tiles + AllGather
    launch, so the collective pipeline starts right after the first pass
    instead of after the whole layer.  SpMM2 runs as TWO passes (c1+c2,
    then c0) consuming j-pairs in AG-arrival order [c1, c2, c0].
  - A's c1+c2 column chunks (7.9 MB/core) stay resident in SBUF across
    both layers; only the c0 chunk is re-streamed in layer 2.  Layer-2's
    final pass reads A entirely from SBUF.
  - GEMM1's PSUM->SBUF casts alternate between DVE and Pool engines (the
    casts, not the matmuls, gated GEMM1 at ~690ns/tile).  SpMM evacs are
    likewise split, except where a pending collective could block the
    Pool queue.
  - x is loaded in 4-j-tile batches; t2 chunks load with one DMA per
    (chunk, peer core) via a rearranged AP; DMA queues are kept separate
    (sync: x/t2/ag_in/out, scalar: A stream, gpsimd: collectives only).
  - A tiny warmup AllGather runs during GEMM1 to absorb the first-op
    collective overhead before AG1 hits the critical path.

All matmuls use perf_mode=DoubleRow (256 contraction rows per matmul):
lhsT/rhs are [128, 2, free] pair tiles, element [p, q] = contraction row
q*128+p.  The transposed SpMM (z^T = t^T A^T) makes each layer's
activation land in [feature, node] layout, exactly the lhsT the next
GEMM needs -- no on-device transposes anywhere.
"""

import contextlib
import ctypes
import sys
import types

import ml_dtypes
import numpy as np

import concourse.bass as bass
import concourse.mybir as mybir
import concourse.tile as tile
from concourse.bass_utils import run_bass_kernel_spmd

BF16 = ml_dtypes.bfloat16
FP8 = ml_dtypes.float8_e4m3

N_CORES = 8
N_NODES = 10000
F_IN = 512
F_HID = 512
N_CLASSES = 16
NP = 10240            # padded node count (80 * 128)
R = NP // N_CORES     # 1280 rows per core
P = 128
NJ = NP // P          # 80 contraction chunks
NJ4 = NJ // 4         # 20 four-tile x groups
NJP = NJ // 2         # 40 DoubleRow contraction pairs
NM = R // P           # 10 row tiles per core
NF = F_HID // P       # 4 feature tiles
NFP = NF // 2         # 2 feature pairs

# dst-column chunks of this core's 1280 columns; AG launch order c1, c2, c0
CH_OFF = {"c0": 0, "c1": 512, "c2": 1024}
CH_W = {"c0": 512, "c1": 512, "c2": 256}
CH_M = {"c0": range(0, 4), "c1": range(4, 8), "c2": range(8, 10)}
CH_NT = {"c0": 4, "c1": 4, "c2": 2}     # j-tiles per core per chunk

_NTFF_HOOK_INSTALLED = False


def install_ntff_hook():
    """bass_utils' trace=True path wants antenv.axon_hooks; this container
    doesn't ship it, so provide the same ctypes hook trn_boot would."""
    global _NTFF_HOOK_INSTALLED
    if _NTFF_HOOK_INSTALLED:
        return
    _NTFF_HOOK_INSTALLED = True
    try:
        lib = ctypes.CDLL("/opt/axon/libaxon_pjrt.so")
        if not hasattr(lib, "axon_start_nrt_profile"):
            return
    except OSError:
        return
    lib.axon_start_nrt_profile.argtypes = [
        ctypes.POINTER(ctypes.c_int64),
        ctypes.c_size_t,
    ]
    lib.axon_start_nrt_profile.restype = ctypes.c_int64
    lib.axon_stop_nrt_profile.argtypes = [ctypes.c_char_p]
    lib.axon_stop_nrt_profile.restype = ctypes.c_int64

    @contextlib.contextmanager
    def _hook(output_dir, device_ids):
        import jax

        jax.devices()
        if device_ids:
            ids = (ctypes.c_int64 * len(device_ids))(*device_ids)
            rc = lib.axon_start_nrt_profile(ids, len(device_ids))
        else:
            rc = lib.axon_start_nrt_profile(None, 0)
        if rc != 0:
            raise RuntimeError(f"axon_start_nrt_profile rc={rc}")
        try:
            yield
        finally:
            n = lib.axon_stop_nrt_profile(str(output_dir).encode())
            print(f"ntff profile: {n} file(s) -> {output_dir}", file=sys.stderr)

    import antenv

    mod = types.ModuleType("antenv.axon_hooks")
    mod.get_axon_ntff_profile_hook = lambda: _hook
    mod.set_axon_ntff_profile_hook = lambda h: None
    sys.modules["antenv.axon_hooks"] = mod
    antenv.axon_hooks = mod


def split_drain_waits(nc):
    """This walrus build allows only ONE sync-wait per lowered instruction
    (CTRL and pseudo-DMA structs assert on more).  Tile's wait-assignment can
    attach several; keep the last wait on the instruction and move the rest
    onto preceding single-wait NoOps on the same engine stream (waits are
    monotonic >= conditions, so enforcing them earlier in program order on
    the same engine is equivalent)."""
    for f in nc.m.functions:
        for bb in f.blocks:
            insts = bb.instructions
            i = 0
            while i < len(insts):
                inst = insts[i]
                si = getattr(inst, "sync_info", None)
                if si is not None and si.on_wait and len(si.on_wait) > 1:
                    waits = list(si.on_wait)
                    si.on_wait = [waits[-1]]
                    for j, w in enumerate(waits[:-1]):
                        pre = mybir.InstNoOp(
                            name=f"{inst.name}-presync-{j}",
                            engine=inst.engine,
                            ins=[],
                            outs=[],
                            sync_info=mybir.SyncInfo(on_wait=[w], on_update=[]),
                        )
                        insts.insert(i + j, pre)
                        nc.register_instruction(pre, overwrite=True)
                    i += len(waits) - 1
                i += 1


def build_gcn(nc):
    """Emit the SPMD GCN program (identical on every core; per-core data)."""
    f32 = mybir.dt.float32
    bf16 = mybir.dt.bfloat16
    fp8 = mybir.dt.float8e4
    DR = mybir.MatmulPerfMode.DoubleRow
    rg = [list(range(N_CORES))]

    # I/O (per-core shards; same names on every core)
    # xTt84[g, p, q, cq, m] = x[(4g+q)*128+m, cq*128+p]  (fp8)
    xTt84 = nc.declare_dram_parameter("xTt84", [NJ4, P, 4, NF, P], fp8, isOutput=False)
    # ATdr[jp, p, q, m] = AhatT[jp*256 + q*128 + p, k*R + m]  (fp8 pairs)
    ATdr = nc.declare_dram_parameter("ATdr", [NJP, P, 2, R], fp8, isOutput=False)
    # W pair layouts: W*p8[t, p, q, n] = W[(2t+q)*128 + p, n]
    W1p = nc.declare_dram_parameter("W1p", [NFP, P, 2, F_HID], fp8, isOutput=False)
    W2p = nc.declare_dram_parameter("W2p", [NFP, P, 2, F_HID], fp8, isOutput=False)
    Woutp = nc.declare_dram_parameter("Woutp", [NFP, P, 2, N_CLASSES], fp8, isOutput=False)
    bcols = nc.declare_dram_parameter("bcols", [P, 2 * NF], f32, isOutput=False)
    bout = nc.declare_dram_parameter("bout", [1, N_CLASSES], bf16, isOutput=False)
    out = nc.declare_dram_parameter("out", [R, N_CLASSES], f32, isOutput=True)

    # layer-2 collective bounce buffers (internal DRAM), 3 chunks, fp8
    ag_in = nc.dram_tensor("ag_in", [R, F_HID], fp8)
    ag_out = {
        c: nc.dram_tensor(
            f"ag_out_{c}", [N_CORES * CH_NT[c], P, F_HID], fp8, addr_space="Shared"
        )
        for c in ("c0", "c1", "c2")
    }
    ag_wu_in = nc.dram_tensor("ag_wu_in", [P, 64], fp8)
    ag_wu_out = nc.dram_tensor("ag_wu_out", [N_CORES * P, 64], fp8, addr_space="Shared")

    with tile.TileContext(nc) as tc:
        with (
            tc.tile_pool(name="const", bufs=1) as cpool,
            tc.tile_pool(name="ares", bufs=1) as apool,
            tc.tile_pool(name="tfull", bufs=1) as tpool,
            tc.tile_pool(name="hT", bufs=1) as hpool,
            tc.tile_pool(name="work", bufs=4) as wpool,
            tc.tile_pool(name="evac", bufs=4) as epool,
            tc.tile_pool(name="sm", bufs=4) as spool,
            tc.tile_pool(name="psum", bufs=1, space="PSUM") as ppool,
        ):
            # ---- GEMM1 constants (needed immediately) ----
            W1_sb = [cpool.tile([P, 2, F_HID], fp8, tag=f"W1{t}", name=f"W1{t}") for t in range(NFP)]
            for t in range(NFP):
                nc.sync.dma_start(out=W1_sb[t][:], in_=W1p[t, :, :, :])

            # warmup AG feed (trigger is emitted after GEMM1's pool casts)
            wu_sb = cpool.tile([P, 64], fp8, tag="wu", name="wu")
            nc.vector.memset(wu_sb[:], 0.0)
            nc.sync.dma_start(out=ag_wu_in[:, :], in_=wu_sb[:])

            # persistent activation tiles
            # layer 1: j-PAIR tiles  t1p[jp][p, q, f] = t1[jp*256+q*128+p... ]
            t1p = [
                tpool.tile([P, 2, F_HID], fp8, tag=f"t1_{jp}", name=f"t1_{jp}")
                for jp in range(NJP)
            ]
            # layer 2: per (chunk, peer core) tiles holding that core's
            # chunk j-tiles: t2c[c][r][p, i, f] = t2[(r*NM + base + i)*128 + p, f]
            t2c = {
                c: [
                    tpool.tile([P, CH_NT[c], F_HID], fp8, tag=f"t2{c}_{r}", name=f"t2{c}_{r}")
                    for r in range(N_CORES)
                ]
                for c in ("c0", "c1", "c2")
            }
            # hT as fp8 feature-pair tiles: hp[layer][t][p, q, m], ft = 2t+q
            hp = [
                [hpool.tile([P, 2, R], fp8, tag=f"h{la}p{t}", name=f"h{la}p{t}") for t in range(NFP)]
                for la in range(2)
            ]
            # resident A chunks (loaded once up front, reused in both layers);
            # the loads have no deps, so the scalar DMA ring streams them all
            # during GEMM1, well ahead of the SpMM passes that consume them.
            atc1 = [
                apool.tile([P, 2, CH_W["c1"]], fp8, tag=f"ac1_{jp}", name=f"ac1_{jp}")
                for jp in range(NJP)
            ]
            atc2 = [
                apool.tile([P, 2, CH_W["c2"]], fp8, tag=f"ac2_{jp}", name=f"ac2_{jp}")
                for jp in range(NJP)
            ]
            def load_res(eng, tiles, chunk, jp):
                off, w = CH_OFF[chunk], CH_W[chunk]
                eng.dma_start(out=tiles[jp][:], in_=ATdr[jp, :, :, off:off + w])

            # ---- layer 1: replicated GEMM1 (fp8 DoubleRow) ----
            # x loads alternate between the SP and gpsimd DMA rings: one ring
            # (~95 GB/s) cannot feed GEMM1's ~150 GB/s appetite
            for g in range(NJ4):
                xt4 = wpool.tile([P, 4, NF, P], fp8, tag="xt4", name="xt4")
                xq = nc.sync if g % 2 == 0 else nc.gpsimd
                xq.dma_start(out=xt4[:], in_=xTt84[g, :, :, :, :])
                for q in range(4):
                    j = 4 * g + q
                    # 8-bank rotation: with 4 banks the mm->cast->mm chain
                    # (two cross-engine semaphore hops per lap) paces GEMM1
                    ps = ppool.tile([P, F_HID], f32, tag=f"sp{j % 8}", name=f"g1ps{j % 8}")
                    for t in range(NFP):
                        nc.tensor.matmul(
                            out=ps[:],
                            lhsT=xt4[:, q, 2 * t:2 * t + 2, :],
                            rhs=W1_sb[t][:, :, :],
                            start=(t == 0),
                            stop=(t == NFP - 1),
                            perf_mode=DR,
                        )
                    # casts split across DVE/Act; one resident-A load per j,
                    # interleaved so neither DMA ring backs up its engine
                    # queue (a bulk 40-load block stalls the queue -- and the
                    # casts behind it -- on descriptor-ring space)
                    if j % 2 == 0:
                        nc.vector.tensor_copy(out=t1p[j // 2][:, j % 2, :], in_=ps[:])
                    else:
                        nc.scalar.activation(
                            out=t1p[j // 2][:, j % 2, :], in_=ps[:],
                            func=mybir.ActivationFunctionType.Copy,
                        )
                        load_res(nc.scalar, atc1, "c1", j // 2)

            # warmup collective: absorbs first-op overhead during GEMM1
            nc.gpsimd.collective_compute(
                "AllGather",
                mybir.AluOpType.bypass,
                replica_groups=rg,
                ins=[ag_wu_in[:, :].opt()],
                outs=[ag_wu_out[:, :].opt()],
            )

            # c2 resident loads: bulk on the gpsimd ring after GEMM1's x
            # loads (they are only needed by the final s1c2 pass ~80us out)
            for jp in range(NJP):
                load_res(nc.gpsimd, atc2, "c2", jp)

            # ---- deferred constants (needed after GEMM1 starts) ----
            W2_sb = [cpool.tile([P, 2, F_HID], fp8, tag=f"W2{t}", name=f"W2{t}") for t in range(NFP)]
            for t in range(NFP):
                nc.sync.dma_start(out=W2_sb[t][:], in_=W2p[t, :, :, :])
            Wout_sb = [cpool.tile([P, 2, N_CLASSES], fp8, tag=f"Wo{t}", name=f"Wo{t}") for t in range(NFP)]
            for t in range(NFP):
                nc.sync.dma_start(out=Wout_sb[t][:], in_=Woutp[t, :, :, :])
            bcols_sb = cpool.tile([P, 2 * NF], f32, tag="bcols", name="bcols")
            nc.sync.dma_start(out=bcols_sb[:], in_=bcols[:, :])
            bout_sb = cpool.tile([1, N_CLASSES], bf16, tag="bout", name="bout")
            nc.sync.dma_start(out=bout_sb[:], in_=bout[:, :])
            ones_sb = cpool.tile([1, P], bf16, tag="ones", name="ones")
            nc.vector.memset(ones_sb[:], 1.0)

            # ---- helpers ----
            def src1(jp):
                return t1p[jp]

            def src2(jp):
                r, k = jp // 5, jp % 5
                if k < 2:
                    return t2c["c0"][r][:, 2 * k:2 * k + 2, :]
                if k < 4:
                    return t2c["c1"][r][:, 2 * (k - 2):2 * (k - 2) + 2, :]
                return t2c["c2"][r][:, 0:2, :]

            def a_res(tiles):
                return lambda jp: tiles[jp]

            AT0_BUFS = 12
            at0_pending = {}

            def a_stream_load(jp):
                at = wpool.tile(
                    [P, 2, CH_W["c0"]], fp8, tag="at0", name="at0", bufs=AT0_BUFS
                )
                nc.scalar.dma_start(
                    out=at[:],
                    in_=ATdr[jp, :, :, CH_OFF["c0"]:CH_OFF["c0"] + CH_W["c0"]],
                )
                at0_pending[jp] = at

            def a_stream(jp_order):
                """Streamed c0 A tiles, prefetched AT0_BUFS deep.  The load
                for jp_order[i+BUFS] is emitted via post() AFTER jp_order[i]'s
                matmuls so the ring-buffer WAR (writer of slot i+BUFS after
                readers of slot i) follows program order."""
                for jp in jp_order[:AT0_BUFS]:
                    a_stream_load(jp)

                def get(jp):
                    return at0_pending.pop(jp)

                def post(idx):
                    k = idx + AT0_BUFS
                    if k < len(jp_order):
                        a_stream_load(jp_order[k])

                return get, post

            def spmm_pass(name, layer, src_fn, chunks, jp_order, post=None):
                """One fp8 DoubleRow accumulation sweep over the given dst
                chunks of hT[layer] = relu(t^T A^T + b).
                chunks: list of (chunk_key, bank_base, a_fn)."""
                pst = {}
                for (ck, bb, _a) in chunks:
                    for f in range(NF):
                        pst[(ck, f)] = ppool.tile(
                            [P, CH_W[ck]], f32, tag=f"sp{bb + f}", name=f"{name}_{bb + f}"
                        )
                last = len(jp_order) - 1
                for idx, jp in enumerate(jp_order):
                    ats = {ck: a_fn(jp) for (ck, bb, a_fn) in chunks}
                    src = src_fn(jp)
                    for f in range(NF):
                        for (ck, bb, _a) in chunks:
                            nc.tensor.matmul(
                                out=pst[(ck, f)][:],
                                lhsT=src[:, :, f * P:(f + 1) * P],
                                rhs=ats[ck][:, :, :],
                                start=(idx == 0),
                                stop=(idx == last),
                                perf_mode=DR,
                            )
                    if post is not None:
                        post(idx)
                # evacuate: relu(psum + b) -> fp8 pair tiles; f-tile ft=2t+q
                # (split across DVE and Activation so neither engine gates the
                # downstream GEMM2 + AllGather launch)
                for (ck, bb, _a) in chunks:
                    off, w = CH_OFF[ck], CH_W[ck]
                    for f in range(NF):
                        bc = bcols_sb[:, layer * NF + f:layer * NF + f + 1]
                        if f % 2 == 0:
                            nc.vector.tensor_scalar(
                                out=hp[layer][f // 2][:, f % 2, off:off + w],
                                in0=pst[(ck, f)][:],
                                scalar1=bc,
                                scalar2=0.0,
                                op0=mybir.AluOpType.add,
                                op1=mybir.AluOpType.max,
                            )
                        else:
                            nc.scalar.activation(
                                out=hp[layer][f // 2][:, f % 2, off:off + w],
                                in_=pst[(ck, f)][:],
                                func=mybir.ActivationFunctionType.Relu,
                                bias=bc,
                            )

            def gemm2_tiles(ms, bb=0):
                """t2_k rows for m-tiles `ms` staged into ag_in (as fp8)."""
                for m in ms:
                    ps = ppool.tile([P, F_HID], f32, tag=f"sp{bb + m % 4}", name=f"g2ps{bb + m % 4}")
                    for t in range(NFP):
                        nc.tensor.matmul(
                            out=ps[:],
                            lhsT=hp[0][t][:, :, m * P:(m + 1) * P],
                            rhs=W2_sb[t][:, :, :],
                            start=(t == 0),
                            stop=(t == NFP - 1),
                            perf_mode=DR,
                        )
                    ev = epool.tile([P, F_HID], fp8, tag="g2ev", name="g2ev")
                    nc.vector.tensor_copy(out=ev[:], in_=ps[:])
                    nc.sync.dma_start(out=ag_in[m * P:(m + 1) * P, :], in_=ev[:])

            def ag_chunk(c):
                off = CH_OFF[c]
                rows = CH_W[c]
                nc.gpsimd.collective_compute(
                    "AllGather",
                    mybir.AluOpType.bypass,
                    replica_groups=rg,
                    ins=[ag_in[off:off + rows, :].opt()],
                    outs=[ag_out[c][:, :, :].opt()],
                )

            def load_t2_chunk(c):
                nt = CH_NT[c]
                for r in range(N_CORES):
                    eng = nc.sync if r % 2 == 0 else nc.gpsimd
                    eng.dma_start(
                        out=t2c[c][r][:, :, :],
                        in_=ag_out[c][r * nt:(r + 1) * nt, :, :].rearrange(
                            "a b c -> b a c"
                        ),
                    )

            def head_tiles(ms):
                """logits + softmax for m-tiles `ms` of this core."""
                for m in ms:
                    ps = ppool.tile([P, N_CLASSES], f32, tag=f"sp{m % 4}", name=f"hps{m % 4}")
                    for t in range(NFP):
                        nc.tensor.matmul(
                            out=ps[:],
                            lhsT=hp[1][t][:, :, m * P:(m + 1) * P],
                            rhs=Wout_sb[t][:, :, :],
                            start=(t == 0),
                            stop=False,
                            perf_mode=DR,
                        )
                    nc.tensor.matmul(
                        out=ps[:],
                        lhsT=ones_sb[:, 0:P],
                        rhs=bout_sb[:],
                        start=False,
                        stop=True,
                    )
                    negmax = spool.tile([P, 1], f32, tag="negmax", name="negmax")
                    nc.vector.tensor_reduce(
                        out=negmax[:], in_=ps[:], axis=mybir.AxisListType.X,
                        op=mybir.AluOpType.max, negate=True,
                    )
                    ex = spool.tile([P, N_CLASSES], f32, tag="ex", name="ex")
                    nc.scalar.activation(
                        out=ex[:], in_=ps[:],
                        func=mybir.ActivationFunctionType.Exp,
                        bias=negmax[:, 0:1],
                    )
                    ssum = spool.tile([P, 1], f32, tag="ssum", name="ssum")
                    nc.vector.tensor_reduce(
                        out=ssum[:], in_=ex[:], axis=mybir.AxisListType.X,
                        op=mybir.AluOpType.add,
                    )
                    rinv = spool.tile([P, 1], f32, tag="rinv", name="rinv")
                    nc.vector.reciprocal(out=rinv[:], in_=ssum[:])
                    prob = spool.tile([P, N_CLASSES], f32, tag="prob", name="prob")
                    nc.vector.tensor_scalar_mul(prob[:], ex[:], rinv[:, 0:1])
                    nc.sync.dma_start(out=out[m * P:(m + 1) * P, :], in_=prob[:])

            natural = list(range(NJP))
            # SpMM2 consumes j-pairs in AG-arrival order: c1, c0, c2
            order2 = (
                [5 * r + k for r in range(N_CORES) for k in (2, 3)]
                + [5 * r + k for r in range(N_CORES) for k in (0, 1)]
                + [5 * r + 4 for r in range(N_CORES)]
            )
            assert sorted(order2) == natural

            # ---- layer 1 SpMM: 3 chunk passes, GEMM2 + AG after each.
            # The small c2 chunk goes LAST so the final (tail-critical)
            # AllGather is the short one and its pairs are the last SpMM2
            # consumes.
            g1, p1 = a_stream(natural)
            spmm_pass("s1c1", 0, src1, [("c1", 4, a_res(atc1))], natural)
            gemm2_tiles(CH_M["c1"], bb=0)
            ag_chunk("c1")
            spmm_pass("s1c0", 0, src1, [("c0", 4, g1)], natural, post=p1)
            gemm2_tiles(CH_M["c0"], bb=0)
            ag_chunk("c0")
            load_t2_chunk("c1")
            spmm_pass("s1c2", 0, src1, [("c2", 0, a_res(atc2))], natural)
            gemm2_tiles(CH_M["c2"], bb=4)
            ag_chunk("c2")
            load_t2_chunk("c0")
            load_t2_chunk("c2")

            # ---- layer 2 SpMM: c1+c2 pass (resident A), then c0 pass ----
            g2, p2 = a_stream(order2)
            spmm_pass(
                "s2a", 1, src2,
                [("c1", 4, a_res(atc1)), ("c2", 0, a_res(atc2))],
                order2,
            )
            head_tiles(list(CH_M["c1"]) + list(CH_M["c2"]))
            spmm_pass("s2b", 1, src2, [("c0", 4, g2)], order2, post=p2)
            head_tiles(CH_M["c0"])

    return nc


def build_inputs(x, edge_index, W1, b1, W2, b2, Wout, bout):
    """Host-side graph preprocessing + per-core shard construction."""
    x = np.asarray(x)
    ei = np.asarray(edge_index)
    n = N_NODES
    src = np.concatenate([ei[0], np.arange(n, dtype=np.int64)])
    dst = np.concatenate([ei[1], np.arange(n, dtype=np.int64)])
    deg = np.bincount(dst, minlength=n).astype(np.float32)
    dinv = 1.0 / np.sqrt(deg)
    normv = (dinv[src] * dinv[dst]).astype(np.float32)

    # dense Ahat^T, padded:  AhatT[src, dst] = norm  (duplicate edges sum)
    AhatT = np.zeros((NP, NP), dtype=np.float32)
    np.add.at(AhatT, (src, dst), normv)
    # DoubleRow pair-interleave: ATdr[jp, p, q, :] = AhatT[jp*256+q*128+p, :]
    ATdr = np.ascontiguousarray(
        AhatT.reshape(NJP, 2, P, NP).transpose(0, 2, 1, 3)
    ).astype(FP8)

    xp = np.zeros((NP, F_IN), dtype=np.float32)
    xp[:n] = x
    # xTt84[g, p, q, cq, m] = x[(4g+q)*128+m, cq*128+p]
    xTt84 = np.ascontiguousarray(
        xp.reshape(NJ4, 4, P, NF, P).transpose(0, 4, 1, 3, 2)
    ).astype(FP8)

    def wpairs(W):
        W = np.asarray(W, np.float32)
        # [t, p, q, n] = W[(2t+q)*128+p, n]
        return np.ascontiguousarray(
            W.reshape(NFP, 2, P, W.shape[1]).transpose(0, 2, 1, 3)
        ).astype(FP8)

    W1b = wpairs(W1)
    W2b = wpairs(W2)
    Woutb = wpairs(Wout)
    boutb = np.asarray(bout).reshape(1, N_CLASSES).astype(BF16)
    # biases as per-partition columns: bcols[:, l*NF + f] = b_l[f*128:(f+1)*128]
    bcols = np.stack(
        [np.asarray(b1).reshape(NF, P), np.asarray(b2).reshape(NF, P)], 0
    ).reshape(2 * NF, P).T.astype(np.float32)
    bcols = np.ascontiguousarray(bcols)

    in_maps = []
    for k in range(N_CORES):
        sl = slice(k * R, (k + 1) * R)
        in_maps.append({
            "xTt84": xTt84,
            "ATdr": np.ascontiguousarray(ATdr[:, :, :, sl]),
            "W1p": W1b,
            "W2p": W2b,
            "Woutp": Woutb,
            "bcols": bcols,
            "bout": boutb,
        })
    return in_maps


_CACHED = {}


def _get_program():
    if "nc" not in _CACHED:
        nc = bass.Bass(num_devices=N_CORES)
        build_gcn(nc)
        split_drain_waits(nc)
        _CACHED["nc"] = nc
    return _CACHED["nc"]


def kernel(x, edge_index, W1, b1, W2, b2, Wout, bout, trace=False):
    install_ntff_hook()
    nc = _get_program()
    in_maps = build_inputs(x, edge_index, W1, b1, W2, b2, Wout, bout)
    res = run_bass_kernel_spmd(
        nc, in_maps, core_ids=list(range(N_CORES)), trace=trace
    )
    out = np.concatenate([res.results[k]["out"] for k in range(N_CORES)], 0)
    kernel.last_exec_time_ns = res.exec_time_ns
    kernel.last_results = res
    return out[:N_NODES].astype(np.float32)


kernel.last_exec_time_ns = None
kernel.last_results = None


# revision 24
# speedup vs baseline: 1.0767x; 1.0513x over previous
"""Trainium2 8-core GCN kernel (2-layer GCNConv + linear head + softmax).

Strategy (node/row partitioning, dense normalized adjacency):
  - Host: build Ahat = D^-1/2 (A+I) D^-1/2 as a dense fp8-e4m3 matrix, padded
    from 10000 to 10240 nodes; core k owns node rows [k*1280, (k+1)*1280).
  - Device, per core k (all matmuls fp8-e4m3 DoubleRow, fp32 accumulate):
      t1     = x @ W1 for ALL nodes (replicated GEMM; cheaper than the
               all-gather + reload stall it replaces)
      h1T_k  = relu(t1^T Ahat^T[:,k] + b1)    (transposed SpMM -> [512,1280])
      t2_k   = (h1T_k)^T @ W2                 (h1T is directly the lhsT)
      t2     = AllGather(t2_k) in 3 chunks
      h2T_k  = relu(t2^T Ahat^T[:,k] + b2)
      out_k  = softmax(h2T_k^T @ Wout + bout) ([1280, 16] f32)
  - Host: concatenate core outputs, trim padding to [10000, 16].

v2 schedule notes (vs the first working version):
  - SpMM1 runs as THREE dst-chunk passes (c1=cols 512:1024, c2=1024:1280,
    c0=0:512), each immediately followed by its GEMM2 m-tiles + AllGather
    launch, so the collective pipeline starts right after the first pass
    instead of after the whole layer.  SpMM2 runs as TWO passes (c1+c2,
    then c0) consuming j-pairs in AG-arrival order [c1, c2, c0].
  - A's c1+c2 column chunks (7.9 MB/core) stay resident in SBUF across
    both layers; only the c0 chunk is re-streamed in layer 2.  Layer-2's
    final pass reads A entirely from SBUF.
  - GEMM1's PSUM->SBUF casts alternate between DVE and Pool engines (the
    casts, not the matmuls, gated GEMM1 at ~690ns/tile).  SpMM evacs are
    likewise split, except where a pending collective could block the
    Pool queue.
  - x is loaded in 4-j-tile batches; t2 chunks load with one DMA per
    (chunk, peer core) via a rearranged AP; DMA queues are kept separate
    (sync: x/t2/ag_in/out, scalar: A stream, gpsimd: collectives only).
  - A tiny warmup AllGather runs during GEMM1 to absorb the first-op
    collective overhead before AG1 hits the critical path.

All matmuls use perf_mode=DoubleRow (256 contraction rows per matmul):
lhsT/rhs are [128, 2, free] pair tiles, element [p, q] = contraction row
q*128+p.  The transposed SpMM (z^T = t^T A^T) makes each layer's
activation land in [feature, node] layout, exactly the lhsT the next
GEMM needs -- no on-device transposes anywhere.
"""

import contextlib
import ctypes
import sys
import types

import ml_dtypes
import numpy as np

import concourse.bass as bass
import concourse.mybir as mybir
import concourse.tile as tile
from concourse.bass_utils import run_bass_kernel_spmd

BF16 = ml_dtypes.bfloat16
FP8 = ml_dtypes.float8_e4m3

N_CORES = 8
N_NODES = 10000
F_IN = 512
F_HID = 512
N_CLASSES = 16
NP = 10240            # padded node count (80 * 128)
R = NP // N_CORES     # 1280 rows per core
P = 128
NJ = NP // P          # 80 contraction chunks
NJ4 = NJ // 4         # 20 four-tile x groups
NJP = NJ // 2         # 40 DoubleRow contraction pairs
NM = R // P           # 10 row tiles per core
NF = F_HID // P       # 4 feature tiles
NFP = NF // 2         # 2 feature pairs

# dst-column chunks of this core's 1280 columns; AG launch order c1, c2, c0
CH_OFF = {"c0": 0, "c1": 512, "c2": 1024}
CH_W = {"c0": 512, "c1": 512, "c2": 256}
CH_M = {"c0": range(0, 4), "c1": range(4, 8), "c2": range(8, 10)}
CH_NT = {"c0": 4, "c1": 4, "c2": 2}     # j-tiles per core per chunk

_NTFF_HOOK_INSTALLED = False


def install_ntff_hook():
    """bass_utils' trace=True path wants antenv.axon_hooks; this container
    doesn't ship it, so provide the same ctypes hook trn_boot would."""
    global _NTFF_HOOK_INSTALLED
    if _NTFF_HOOK_INSTALLED:
        return
    _NTFF_HOOK_INSTALLED = True
    try:
        lib = ctypes.CDLL("/opt/axon/libaxon_pjrt.so")
        if not hasattr(lib, "axon_start_nrt_profile"):
            return
    except OSError:
        return
    lib.axon_start_nrt_profile.argtypes = [
        ctypes.POINTER(ctypes.c_int64),
        ctypes.c_size_t,
    ]
    lib.axon_start_nrt_profile.restype = ctypes.c_int64
    lib.axon_stop_nrt_profile.argtypes = [ctypes.c_char_p]
    lib.axon_stop_nrt_profile.restype = ctypes.c_int64

    @contextlib.contextmanager
    def _hook(output_dir, device_ids):
        import jax

        jax.devices()
        if device_ids:
            ids = (ctypes.c_int64 * len(device_ids))(*device_ids)
            rc = lib.axon_start_nrt_profile(ids, len(device_ids))
        else:
            rc = lib.axon_start_nrt_profile(None, 0)
        if rc != 0:
            raise RuntimeError(f"axon_start_nrt_profile rc={rc}")
        try:
            yield
        finally:
            n = lib.axon_stop_nrt_profile(str(output_dir).encode())
            print(f"ntff profile: {n} file(s) -> {output_dir}", file=sys.stderr)

    import antenv

    mod = types.ModuleType("antenv.axon_hooks")
    mod.get_axon_ntff_profile_hook = lambda: _hook
    mod.set_axon_ntff_profile_hook = lambda h: None
    sys.modules["antenv.axon_hooks"] = mod
    antenv.axon_hooks = mod


def split_drain_waits(nc):
    """This walrus build allows only ONE sync-wait per lowered instruction
    (CTRL and pseudo-DMA structs assert on more).  Tile's wait-assignment can
    attach several; keep the last wait on the instruction and move the rest
    onto preceding single-wait NoOps on the same engine stream (waits are
    monotonic >= conditions, so enforcing them earlier in program order on
    the same engine is equivalent)."""
    for f in nc.m.functions:
        for bb in f.blocks:
            insts = bb.instructions
            i = 0
            while i < len(insts):
                inst = insts[i]
                si = getattr(inst, "sync_info", None)
                if si is not None and si.on_wait and len(si.on_wait) > 1:
                    waits = list(si.on_wait)
                    si.on_wait = [waits[-1]]
                    for j, w in enumerate(waits[:-1]):
                        pre = mybir.InstNoOp(
                            name=f"{inst.name}-presync-{j}",
                            engine=inst.engine,
                            ins=[],
                            outs=[],
                            sync_info=mybir.SyncInfo(on_wait=[w], on_update=[]),
                        )
                        insts.insert(i + j, pre)
                        nc.register_instruction(pre, overwrite=True)
                    i += len(waits) - 1
                i += 1


def build_gcn(nc):
    """Emit the SPMD GCN program (identical on every core; per-core data)."""
    f32 = mybir.dt.float32
    bf16 = mybir.dt.bfloat16
    fp8 = mybir.dt.float8e4
    DR = mybir.MatmulPerfMode.DoubleRow
    rg = [list(range(N_CORES))]

    # I/O (per-core shards; same names on every core)
    # xTt84[g, p, q, cq, m] = x[(4g+q)*128+m, cq*128+p]  (fp8)
    xTt84 = nc.declare_dram_parameter("xTt84", [NJ4, P, 4, NF, P], fp8, isOutput=False)
    # ATdr[jp, p, q, m] = AhatT[jp*256 + q*128 + p, k*R + m]  (fp8 pairs)
    ATdr = nc.declare_dram_parameter("ATdr", [NJP, P, 2, R], fp8, isOutput=False)
    # W pair layouts: W*p8[t, p, q, n] = W[(2t+q)*128 + p, n]
    W1p = nc.declare_dram_parameter("W1p", [NFP, P, 2, F_HID], fp8, isOutput=False)
    W2p = nc.declare_dram_parameter("W2p", [NFP, P, 2, F_HID], fp8, isOutput=False)
    Woutp = nc.declare_dram_parameter("Woutp", [NFP, P, 2, N_CLASSES], fp8, isOutput=False)
    bcols = nc.declare_dram_parameter("bcols", [P, 2 * NF], f32, isOutput=False)
    bout = nc.declare_dram_parameter("bout", [1, N_CLASSES], bf16, isOutput=False)
    out = nc.declare_dram_parameter("out", [R, N_CLASSES], f32, isOutput=True)

    # layer-2 collective bounce buffers (internal DRAM), 3 chunks, fp8
    ag_in = nc.dram_tensor("ag_in", [R, F_HID], fp8)
    ag_out = {
        c: nc.dram_tensor(
            f"ag_out_{c}", [N_CORES * CH_NT[c], P, F_HID], fp8, addr_space="Shared"
        )
        for c in ("c0", "c1", "c2")
    }
    ag_wu_in = nc.dram_tensor("ag_wu_in", [P, 64], fp8)
    ag_wu_out = nc.dram_tensor("ag_wu_out", [N_CORES * P, 64], fp8, addr_space="Shared")

    with tile.TileContext(nc) as tc:
        with (
            tc.tile_pool(name="const", bufs=1) as cpool,
            tc.tile_pool(name="ares", bufs=1) as apool,
            tc.tile_pool(name="tfull", bufs=1) as tpool,
            tc.tile_pool(name="hT", bufs=1) as hpool,
            tc.tile_pool(name="work", bufs=4) as wpool,
            tc.tile_pool(name="evac", bufs=4) as epool,
            tc.tile_pool(name="sm", bufs=4) as spool,
            tc.tile_pool(name="psum", bufs=1, space="PSUM") as ppool,
        ):
            # ---- GEMM1 constants (needed immediately) ----
            W1_sb = [cpool.tile([P, 2, F_HID], fp8, tag=f"W1{t}", name=f"W1{t}") for t in range(NFP)]
            for t in range(NFP):
                nc.sync.dma_start(out=W1_sb[t][:], in_=W1p[t, :, :, :])

            # warmup AG feed (trigger is emitted after GEMM1's pool casts)
            wu_sb = cpool.tile([P, 64], fp8, tag="wu", name="wu")
            nc.vector.memset(wu_sb[:], 0.0)
            nc.sync.dma_start(out=ag_wu_in[:, :], in_=wu_sb[:])

            # persistent activation tiles
            # layer 1: j-PAIR tiles  t1p[jp][p, q, f] = t1[jp*256+q*128+p... ]
            t1p = [
                tpool.tile([P, 2, F_HID], fp8, tag=f"t1_{jp}", name=f"t1_{jp}")
                for jp in range(NJP)
            ]
            # layer 2: per (chunk, peer core) tiles holding that core's
            # chunk j-tiles: t2c[c][r][p, i, f] = t2[(r*NM + base + i)*128 + p, f]
            t2c = {
                c: [
                    tpool.tile([P, CH_NT[c], F_HID], fp8, tag=f"t2{c}_{r}", name=f"t2{c}_{r}")
                    for r in range(N_CORES)
                ]
                for c in ("c0", "c1", "c2")
            }
            # hT as fp8 feature-pair tiles: hp[layer][t][p, q, m], ft = 2t+q
            hp = [
                [hpool.tile([P, 2, R], fp8, tag=f"h{la}p{t}", name=f"h{la}p{t}") for t in range(NFP)]
                for la in range(2)
            ]
            # resident A chunks (loaded once up front, reused in both layers);
            # the loads have no deps, so the scalar DMA ring streams them all
            # during GEMM1, well ahead of the SpMM passes that consume them.
            atc1 = [
                apool.tile([P, 2, CH_W["c1"]], fp8, tag=f"ac1_{jp}", name=f"ac1_{jp}")
                for jp in range(NJP)
            ]
            atc2 = [
                apool.tile([P, 2, CH_W["c2"]], fp8, tag=f"ac2_{jp}", name=f"ac2_{jp}")
                for jp in range(NJP)
            ]
            def load_res(eng, tiles, chunk, jp):
                off, w = CH_OFF[chunk], CH_W[chunk]
                eng.dma_start(out=tiles[jp][:], in_=ATdr[jp, :, :, off:off + w])

            # ---- layer 1: replicated GEMM1 (fp8 DoubleRow) ----
            # x loads alternate between the SP and gpsimd DMA rings: one ring
            # (~95 GB/s) cannot feed GEMM1's ~150 GB/s appetite
            for g in range(NJ4):
                # bufs=10: the front window oversubscribes aggregate DMA
                # bandwidth (x + resident-A prefetch), so GEMM1 needs a deep
                # x prefetch to ride out ring backlog
                xt4 = wpool.tile([P, 4, NF, P], fp8, tag="xt4", name="xt4", bufs=10)
                xq = nc.sync if g % 2 == 0 else nc.gpsimd
                xq.dma_start(out=xt4[:], in_=xTt84[g, :, :, :, :])
                for q in range(4):
                    j = 4 * g + q
                    # 8-bank rotation: with 4 banks the mm->cast->mm chain
                    # (two cross-engine semaphore hops per lap) paces GEMM1
                    ps = ppool.tile([P, F_HID], f32, tag=f"sp{j % 8}", name=f"g1ps{j % 8}")
                    for t in range(NFP):
                        nc.tensor.matmul(
                            out=ps[:],
                            lhsT=xt4[:, q, 2 * t:2 * t + 2, :],
                            rhs=W1_sb[t][:, :, :],
                            start=(t == 0),
                            stop=(t == NFP - 1),
                            perf_mode=DR,
                        )
                    # casts split across DVE/Act; one resident-A load per j,
                    # interleaved so neither DMA ring backs up its engine
                    # queue (a bulk 40-load block stalls the queue -- and the
                    # casts behind it -- on descriptor-ring space)
                    if j % 2 == 0:
                        nc.vector.tensor_copy(out=t1p[j // 2][:, j % 2, :], in_=ps[:])
                    else:
                        nc.scalar.activation(
                            out=t1p[j // 2][:, j % 2, :], in_=ps[:],
                            func=mybir.ActivationFunctionType.Copy,
                        )
                        load_res(nc.scalar, atc1, "c1", j // 2)

            # warmup collective: absorbs first-op overhead during GEMM1
            nc.gpsimd.collective_compute(
                "AllGather",
                mybir.AluOpType.bypass,
                replica_groups=rg,
                ins=[ag_wu_in[:, :].opt()],
                outs=[ag_wu_out[:, :].opt()],
            )

            # c2 resident loads: bulk on the gpsimd ring after GEMM1's x
            # loads (they are only needed by the final s1c2 pass ~80us out)
            for jp in range(NJP):
                load_res(nc.gpsimd, atc2, "c2", jp)

            # ---- deferred constants (needed after GEMM1 starts) ----
            W2_sb = [cpool.tile([P, 2, F_HID], fp8, tag=f"W2{t}", name=f"W2{t}") for t in range(NFP)]
            for t in range(NFP):
                nc.sync.dma_start(out=W2_sb[t][:], in_=W2p[t, :, :, :])
            Wout_sb = [cpool.tile([P, 2, N_CLASSES], fp8, tag=f"Wo{t}", name=f"Wo{t}") for t in range(NFP)]
            for t in range(NFP):
                nc.sync.dma_start(out=Wout_sb[t][:], in_=Woutp[t, :, :, :])
            bcols_sb = cpool.tile([P, 2 * NF], f32, tag="bcols", name="bcols")
            nc.sync.dma_start(out=bcols_sb[:], in_=bcols[:, :])
            bout_sb = cpool.tile([1, N_CLASSES], bf16, tag="bout", name="bout")
            nc.sync.dma_start(out=bout_sb[:], in_=bout[:, :])
            ones_sb = cpool.tile([1, P], bf16, tag="ones", name="ones")
            nc.vector.memset(ones_sb[:], 1.0)

            # ---- helpers ----
            def src1(jp):
                return t1p[jp]

            def src2(jp):
                r, k = jp // 5, jp % 5
                if k < 2:
                    return t2c["c0"][r][:, 2 * k:2 * k + 2, :]
                if k < 4:
                    return t2c["c1"][r][:, 2 * (k - 2):2 * (k - 2) + 2, :]
                return t2c["c2"][r][:, 0:2, :]

            def a_res(tiles):
                return lambda jp: tiles[jp]

            AT0_BUFS = 12
            at0_pending = {}

            def a_stream_load(jp):
                at = wpool.tile(
                    [P, 2, CH_W["c0"]], fp8, tag="at0", name="at0", bufs=AT0_BUFS
                )
                nc.scalar.dma_start(
                    out=at[:],
                    in_=ATdr[jp, :, :, CH_OFF["c0"]:CH_OFF["c0"] + CH_W["c0"]],
                )
                at0_pending[jp] = at

            def a_stream(jp_order):
                """Streamed c0 A tiles, prefetched AT0_BUFS deep.  The load
                for jp_order[i+BUFS] is emitted via post() AFTER jp_order[i]'s
                matmuls so the ring-buffer WAR (writer of slot i+BUFS after
                readers of slot i) follows program order."""
                for jp in jp_order[:AT0_BUFS]:
                    a_stream_load(jp)

                def get(jp):
                    return at0_pending.pop(jp)

                def post(idx):
                    k = idx + AT0_BUFS
                    if k < len(jp_order):
                        a_stream_load(jp_order[k])

                return get, post

            def spmm_pass(name, layer, src_fn, chunks, jp_order, post=None):
                """One fp8 DoubleRow accumulation sweep over the given dst
                chunks of hT[layer] = relu(t^T A^T + b).
                chunks: list of (chunk_key, bank_base, a_fn)."""
                pst = {}
                for (ck, bb, _a) in chunks:
                    for f in range(NF):
                        pst[(ck, f)] = ppool.tile(
                            [P, CH_W[ck]], f32, tag=f"sp{bb + f}", name=f"{name}_{bb + f}"
                        )
                last = len(jp_order) - 1
                for idx, jp in enumerate(jp_order):
                    ats = {ck: a_fn(jp) for (ck, bb, a_fn) in chunks}
                    src = src_fn(jp)
                    for f in range(NF):
                        for (ck, bb, _a) in chunks:
                            nc.tensor.matmul(
                                out=pst[(ck, f)][:],
                                lhsT=src[:, :, f * P:(f + 1) * P],
                                rhs=ats[ck][:, :, :],
                                start=(idx == 0),
                                stop=(idx == last),
                                perf_mode=DR,
                            )
                    if post is not None:
                        post(idx)
                # evacuate: relu(psum + b) -> fp8 pair tiles; f-tile ft=2t+q
                # (split across DVE and Activation so neither engine gates the
                # downstream GEMM2 + AllGather launch)
                for (ck, bb, _a) in chunks:
                    off, w = CH_OFF[ck], CH_W[ck]
                    for f in range(NF):
                        bc = bcols_sb[:, layer * NF + f:layer * NF + f + 1]
                        if f % 2 == 0:
                            nc.vector.tensor_scalar(
                                out=hp[layer][f // 2][:, f % 2, off:off + w],
                                in0=pst[(ck, f)][:],
                                scalar1=bc,
                                scalar2=0.0,
                                op0=mybir.AluOpType.add,
                                op1=mybir.AluOpType.max,
                            )
                        else:
                            nc.scalar.activation(
                                out=hp[layer][f // 2][:, f % 2, off:off + w],
                                in_=pst[(ck, f)][:],
                                func=mybir.ActivationFunctionType.Relu,
                                bias=bc,
                            )

            def gemm2_tiles(ms, bb=0):
                """t2_k rows for m-tiles `ms` staged into ag_in (as fp8)."""
                for m in ms:
                    ps = ppool.tile([P, F_HID], f32, tag=f"sp{bb + m % 4}", name=f"g2ps{bb + m % 4}")
                    for t in range(NFP):
                        nc.tensor.matmul(
                            out=ps[:],
                            lhsT=hp[0][t][:, :, m * P:(m + 1) * P],
                            rhs=W2_sb[t][:, :, :],
                            start=(t == 0),
                            stop=(t == NFP - 1),
                            perf_mode=DR,
                        )
                    ev = epool.tile([P, F_HID], fp8, tag="g2ev", name="g2ev")
                    nc.vector.tensor_copy(out=ev[:], in_=ps[:])
                    nc.sync.dma_start(out=ag_in[m * P:(m + 1) * P, :], in_=ev[:])

            def ag_chunk(c):
                off = CH_OFF[c]
                rows = CH_W[c]
                nc.gpsimd.collective_compute(
                    "AllGather",
                    mybir.AluOpType.bypass,
                    replica_groups=rg,
                    ins=[ag_in[off:off + rows, :].opt()],
                    outs=[ag_out[c][:, :, :].opt()],
                )

            def load_t2_chunk(c):
                nt = CH_NT[c]
                for r in range(N_CORES):
                    eng = nc.sync if r % 2 == 0 else nc.gpsimd
                    eng.dma_start(
                        out=t2c[c][r][:, :, :],
                        in_=ag_out[c][r * nt:(r + 1) * nt, :, :].rearrange(
                            "a b c -> b a c"
                        ),
                    )

            def head_tiles(ms):
                """logits + softmax for m-tiles `ms` of this core."""
                for m in ms:
                    ps = ppool.tile([P, N_CLASSES], f32, tag=f"sp{m % 4}", name=f"hps{m % 4}")
                    for t in range(NFP):
                        nc.tensor.matmul(
                            out=ps[:],
                            lhsT=hp[1][t][:, :, m * P:(m + 1) * P],
                            rhs=Wout_sb[t][:, :, :],
                            start=(t == 0),
                            stop=False,
                            perf_mode=DR,
                        )
                    nc.tensor.matmul(
                        out=ps[:],
                        lhsT=ones_sb[:, 0:P],
                        rhs=bout_sb[:],
                        start=False,
                        stop=True,
                    )
                    negmax = spool.tile([P, 1], f32, tag="negmax", name="negmax")
                    nc.vector.tensor_reduce(
                        out=negmax[:], in_=ps[:], axis=mybir.AxisListType.X,
                        op=mybir.AluOpType.max, negate=True,
                    )
                    ex = spool.tile([P, N_CLASSES], f32, tag="ex", name="ex")
                    nc.scalar.activation(
                        out=ex[:], in_=ps[:],
                        func=mybir.ActivationFunctionType.Exp,
                        bias=negmax[:, 0:1],
                    )
                    ssum = spool.tile([P, 1], f32, tag="ssum", name="ssum")
                    nc.vector.tensor_reduce(
                        out=ssum[:], in_=ex[:], axis=mybir.AxisListType.X,
                        op=mybir.AluOpType.add,
                    )
                    rinv = spool.tile([P, 1], f32, tag="rinv", name="rinv")
                    nc.vector.reciprocal(out=rinv[:], in_=ssum[:])
                    prob = spool.tile([P, N_CLASSES], f32, tag="prob", name="prob")
                    nc.vector.tensor_scalar_mul(prob[:], ex[:], rinv[:, 0:1])
                    nc.sync.dma_start(out=out[m * P:(m + 1) * P, :], in_=prob[:])

            natural = list(range(NJP))
            # SpMM2 consumes j-pairs in AG-arrival order: c1, c0, c2
            order2 = (
                [5 * r + k for r in range(N_CORES) for k in (2, 3)]
                + [5 * r + k for r in range(N_CORES) for k in (0, 1)]
                + [5 * r + 4 for r in range(N_CORES)]
            )
            assert sorted(order2) == natural

            # ---- layer 1 SpMM: 3 chunk passes, GEMM2 + AG after each.
            # The small c2 chunk goes LAST so the final (tail-critical)
            # AllGather is the short one and its pairs are the last SpMM2
            # consumes.
            g1, p1 = a_stream(natural)
            spmm_pass("s1c1", 0, src1, [("c1", 4, a_res(atc1))], natural)
            gemm2_tiles(CH_M["c1"], bb=0)
            ag_chunk("c1")
            spmm_pass("s1c0", 0, src1, [("c0", 4, g1)], natural, post=p1)
            gemm2_tiles(CH_M["c0"], bb=0)
            ag_chunk("c0")
            load_t2_chunk("c1")
            spmm_pass("s1c2", 0, src1, [("c2", 0, a_res(atc2))], natural)
            gemm2_tiles(CH_M["c2"], bb=4)
            ag_chunk("c2")
            load_t2_chunk("c0")
            load_t2_chunk("c2")

            # ---- layer 2 SpMM: c1+c2 pass (resident A), then c0 pass ----
            g2, p2 = a_stream(order2)
            spmm_pass(
                "s2a", 1, src2,
                [("c1", 4, a_res(atc1)), ("c2", 0, a_res(atc2))],
                order2,
            )
            head_tiles(list(CH_M["c1"]) + list(CH_M["c2"]))
            spmm_pass("s2b", 1, src2, [("c0", 4, g2)], order2, post=p2)
            head_tiles(CH_M["c0"])

    return nc


def build_inputs(x, edge_index, W1, b1, W2, b2, Wout, bout):
    """Host-side graph preprocessing + per-core shard construction."""
    x = np.asarray(x)
    ei = np.asarray(edge_index)
    n = N_NODES
    src = np.concatenate([ei[0], np.arange(n, dtype=np.int64)])
    dst = np.concatenate([ei[1], np.arange(n, dtype=np.int64)])
    deg = np.bincount(dst, minlength=n).astype(np.float32)
    dinv = 1.0 / np.sqrt(deg)
    normv = (dinv[src] * dinv[dst]).astype(np.float32)

    # dense Ahat^T, padded:  AhatT[src, dst] = norm  (duplicate edges sum)
    AhatT = np.zeros((NP, NP), dtype=np.float32)
    np.add.at(AhatT, (src, dst), normv)
    # DoubleRow pair-interleave: ATdr[jp, p, q, :] = AhatT[jp*256+q*128+p, :]
    ATdr = np.ascontiguousarray(
        AhatT.reshape(NJP, 2, P, NP).transpose(0, 2, 1, 3)
    ).astype(FP8)

    xp = np.zeros((NP, F_IN), dtype=np.float32)
    xp[:n] = x
    # xTt84[g, p, q, cq, m] = x[(4g+q)*128+m, cq*128+p]
    xTt84 = np.ascontiguousarray(
        xp.reshape(NJ4, 4, P, NF, P).transpose(0, 4, 1, 3, 2)
    ).astype(FP8)

    def wpairs(W):
        W = np.asarray(W, np.float32)
        # [t, p, q, n] = W[(2t+q)*128+p, n]
        return np.ascontiguousarray(
            W.reshape(NFP, 2, P, W.shape[1]).transpose(0, 2, 1, 3)
        ).astype(FP8)

    W1b = wpairs(W1)
    W2b = wpairs(W2)
    Woutb = wpairs(Wout)
    boutb = np.asarray(bout).reshape(1, N_CLASSES).astype(BF16)
    # biases as per-partition columns: bcols[:, l*NF + f] = b_l[f*128:(f+1)*128]
    bcols = np.stack(
        [np.asarray(b1).reshape(NF, P), np.asarray(b2).reshape(NF, P)], 0
    ).reshape(2 * NF, P).T.astype(np.float32)
    bcols = np.ascontiguousarray(bcols)

    in_maps = []
    for k in range(N_CORES):
        sl = slice(k * R, (k + 1) * R)
        in_maps.append({
            "xTt84": xTt84,
            "ATdr": np.ascontiguousarray(ATdr[:, :, :, sl]),
            "W1p": W1b,
            "W2p": W2b,
            "Woutp": Woutb,
            "bcols": bcols,
            "bout": boutb,
        })
    return in_maps


_CACHED = {}


def _get_program():
    if "nc" not in _CACHED:
        nc = bass.Bass(num_devices=N_CORES)
        build_gcn(nc)
        split_drain_waits(nc)
        _CACHED["nc"] = nc
    return _CACHED["nc"]


def kernel(x, edge_index, W1, b1, W2, b2, Wout, bout, trace=False):
    install_ntff_hook()
    nc = _get_program()
    in_maps = build_inputs(x, edge_index, W1, b1, W2, b2, Wout, bout)
    res = run_bass_kernel_spmd(
        nc, in_maps, core_ids=list(range(N_CORES)), trace=trace
    )
    out = np.concatenate([res.results[k]["out"] for k in range(N_CORES)], 0)
    kernel.last_exec_time_ns = res.exec_time_ns
    kernel.last_results = res
    return out[:N_NODES].astype(np.float32)


kernel.last_exec_time_ns = None
kernel.last_results = None
